# revision 1
# baseline (speedup 1.0000x reference)
"""ChildSum TreeLSTM cell on 8 Trainium2 NeuronCores (Bass/Tile).

Strategy (graph-parallel, per the sharding hint):
  - Partition nodes (parents) into 8 contiguous ranges of N/8; each core owns
    the segment-sum + cell update for its parents.
  - Host does INDEX prep only: sort edges by parent, bucket into 512-parent
    blocks, pad each block to a uniform number of 128-edge chunks (SPMD: one
    program on 8 cores), build per-core compacted child tables (halo nodes,
    split into <=32k-row groups for int16 dma_gather indices), pre-transpose x.
  - Device does all FLOP/memory work: batched dma_gather of child h||c rows
    (one 994ns-overhead SWDGE op per 512-parent block instead of per 128-edge
    chunk), one-hot segment-sum on the PE accumulated in PSUM, dense LSTM
    matmuls, sigmoid/tanh elementwise, output store.

Everything on device is computed in TRANSPOSED orientation [feature, node]:
  V[e, 0:128]=h_child, V[e,128:256]=c_child    (640 gathered edges per block)
  P[e, s] = one-hot(slot[e])                   (iota + is_equal on DVE)
  h_sumT[h, s] += V_h^T P                      (PE, N=512 moving dim)
  c_sumT[h, s] += V_c^T P
  fT/iT/oT/uT[h, n] = W^T xT + U^T h_sumT      (PE; W/U natural layout as lhsT)
  biases ride the ACT `bias` operand (per-partition = per-feature).
Output is written transposed [256, npad] and de-transposed on host.
"""

import os
import sys
import time

for _p in ("/opt/trn_rl_repo", "/root/.axon_site/_ro/trn_rl_repo"):
    if os.path.isdir(_p) and _p not in sys.path:
        sys.path.insert(0, _p)

import numpy as np

import concourse.bass as bass
import concourse.tile as tile
from concourse import mybir
from concourse.bass_utils import run_bass_kernel_spmd
from concourse.library_config import mlp as _mlp_library
from concourse.library_overlay import lower_extended_insts
from concourse.vector_clock import ScopedClock

CORES = 8
S = 512          # parents per block (= PSUM bank free dim in fp32)
P128 = 128
MAX_IDX16 = 32000  # max rows per gather table (int16 indices)

F32 = mybir.dt.float32
I16 = mybir.dt.int16
AF = mybir.ActivationFunctionType
ALU = mybir.AluOpType

# ---------------------------------------------------------------------------
# Workarounds: the walrus build in this container accepts at most ONE sync
# wait per instruction. (a) chunk the Tile tail-drain waits onto nops;
# (b) post-pass that hoists extra waits of any instruction onto preceding
# NoOps on the same engine.
# ---------------------------------------------------------------------------

def _drain_and_barrier_chunked(self, tick_clock, wait_clock):
    probe = self.nc.sync.nop()
    wait_clock.add_sem_waits(probe.ins, ScopedClock({None: tick_clock.global_clock}))
    si = probe.ins.sync_info
    waits = list(si.on_wait) if si is not None else []
    if si is not None:
        probe.ins.sync_info = mybir.SyncInfo(on_wait=waits[:1], on_update=list(si.on_update))
    for i in range(1, len(waits)):
        nop = self.nc.sync.nop()
        nop.ins.sync_info = mybir.SyncInfo(on_wait=waits[i:i + 1], on_update=[])
    self.nc.sync.drain()
    self.nc.all_engine_barrier()
    popped = self.nc._tile_sem_poison_stack.pop()
    assert popped is self._sem_poison
    self.nc.clear_and_free_semaphores(list(self.sems.allocated().values()))
    self.nc.all_engine_barrier()


tile.TileContext._drain_and_barrier = _drain_and_barrier_chunked

_WSPLIT_CTR = [0]


def _split_multi_waits(nc):
    n_split = 0
    for f in nc.m.functions:
        for bb in f.blocks:
            insts = list(bb.instructions)
            if not any(
                i.sync_info is not None and i.sync_info.on_wait and len(i.sync_info.on_wait) > 1
                for i in insts
            ):
                continue
            new = []
            for inst in insts:
                si = inst.sync_info
                if si is not None and si.on_wait and len(si.on_wait) > 1:
                    waits = list(si.on_wait)
                    n_split += 1
                    for w in waits[:-1]:
                        _WSPLIT_CTR[0] += 1
                        new.append(
                            mybir.InstNoOp(
                                name=f"I-wsplit-{_WSPLIT_CTR[0]}",
                                engine=inst.engine,
                                debug=inst.debug,
                                ins=[],
                                outs=[],
                                sync_info=mybir.SyncInfo(on_wait=[w], on_update=[]),
                            )
                        )
                    inst.sync_info = mybir.SyncInfo(
                        on_wait=[waits[-1]], on_update=list(si.on_update)
                    )
                new.append(inst)
            bb.instructions = new
    return n_split


# ---------------------------------------------------------------------------
# Host-side index prep
# ---------------------------------------------------------------------------

def _prep(x, h, c, child_idx, parent_idx):
    N = x.shape[0]
    npc = (N + CORES - 1) // CORES            # parents per core
    nb = (npc + S - 1) // S                   # blocks per core
    npad = nb * S
    nbt = CORES * nb                          # total blocks

    parent = np.asarray(parent_idx).astype(np.int64)
    child = np.asarray(child_idx).astype(np.int64)

    # ---- near-LPT parent -> block assignment (bounds every block's edge
    # count near the mean so c_max = ceil(mean/128); the relabeling is free:
    # xT columns and output rows are permuted on the host anyway).
    deg = np.bincount(parent, minlength=N)
    loads = np.zeros(nbt, np.int64)
    pcount = np.zeros(nbt, np.int64)
    gblock = np.empty(N, np.int64)
    for d in range(int(deg.max()), 0, -1):
        members = np.nonzero(deg == d)[0]
        if len(members) == 0:
            continue
        border = np.argsort(loads, kind="stable")
        k = len(members)
        slots_assign = np.tile(border, -(-k // nbt))[:k]
        gblock[members] = slots_assign
        loads += np.bincount(slots_assign, minlength=nbt) * d
        pcount += np.bincount(slots_assign, minlength=nbt)
    # zero-degree parents fill remaining slot capacity exactly
    d0 = np.nonzero(deg == 0)[0]
    cap = S - pcount
    fill = np.repeat(np.arange(nbt), cap)[: len(d0)]
    gblock[d0] = fill
    pcount += np.bincount(fill, minlength=nbt)
    assert pcount.max() <= S, pcount.max()

    # slot of each parent within its block (stable by parent id)
    order_p = np.argsort(gblock, kind="stable")
    counts = np.bincount(gblock, minlength=nbt)
    starts = np.zeros(nbt + 1, np.int64)
    starts[1:] = np.cumsum(counts)
    slot_of = np.empty(N, np.int64)
    slot_of[order_p] = np.arange(N) - starts[gblock[order_p]]

    c_max = max(1, int(np.ceil(loads.max() / P128)))
    nch = nb * c_max                          # chunks per core
    epb = c_max * P128                        # padded edges per block
    spb = (epb + 15) // 16                    # int16 idx columns per block

    hc = np.ascontiguousarray(
        np.concatenate([np.asarray(h), np.asarray(c)], axis=1), dtype=np.float32
    )

    # edges sorted by target block, then by child within block (locality)
    eblock = gblock[parent]
    eorder = np.argsort(eblock, kind="stable")
    se_block = eblock[eorder]
    se_child = child[eorder]
    se_slot = slot_of[parent][eorder]
    eb_starts = np.zeros(nbt + 1, np.int64)
    eb_starts[1:] = np.cumsum(np.bincount(se_block, minlength=nbt))

    core_slots = []
    core_child = []
    for i in range(CORES):
        slots = np.full(nch * P128, -1.0, np.float32)
        gidx = np.zeros(nch * P128, np.int64)
        for b in range(nb):
            gb = i * nb + b
            e0, e1 = eb_starts[gb], eb_starts[gb + 1]
            mm = e1 - e0
            if mm == 0:
                continue
            ch = se_child[e0:e1]
            sl = se_slot[e0:e1].astype(np.float32)
            so = np.argsort(ch, kind="stable")   # gather locality
            off = b * epb
            slots[off:off + mm] = sl[so]
            gidx[off:off + mm] = ch[so]
        core_slots.append(slots)
        core_child.append(gidx)

    # split blocks into G groups so every per-core group table fits int16
    G = 1
    while True:
        nbg = (nb + G - 1) // G
        ok = True
        groups = []  # per core: list of (blocks range, uniq, remapped idx)
        u_max = [0] * G
        for i in range(CORES):
            gi = []
            for g in range(G):
                b0, b1 = g * nbg, min((g + 1) * nbg, nb)
                seg = slice(b0 * epb, b1 * epb)
                sl = core_slots[i][seg]
                ch = core_child[i][seg]
                real = sl >= 0
                uniq = np.unique(ch[real]) if real.any() else np.array([0], np.int64)
                if len(uniq) > MAX_IDX16:
                    ok = False
                    break
                rem = np.searchsorted(uniq, ch)
                rem[~real] = 0
                u_max[g] = max(u_max[g], len(uniq))
                gi.append((b0, b1, uniq, rem))
            if not ok:
                break
            groups.append(gi)
        if ok:
            break
        G += 1

    core_of_p = gblock // nb
    col_of = (gblock % nb) * S + slot_of
    in_maps = []
    assembly = []
    for i in range(CORES):
        im = {}
        idx16 = np.zeros((P128, nb * spb), np.int16)
        for g in range(G):
            b0, b1, uniq, rem = groups[i][g]
            tbl = np.zeros((u_max[g], 256), np.float32)
            tbl[: len(uniq)] = hc[uniq]
            im[f"hc{g}"] = tbl
            for b in range(b0, b1):
                e = rem[(b - b0) * epb:(b - b0 + 1) * epb].astype(np.int16)
                # dma_gather order: edge i = col*16 + partition(p<16); replicate x8
                blk = e.reshape(spb, 16).T          # [16, spb]
                idx16[:, b * spb:(b + 1) * spb] = np.tile(blk, (8, 1))
        im["idx16"] = idx16
        im["slots"] = np.ascontiguousarray(
            core_slots[i].reshape(nch, P128).T)
        pi = np.nonzero(core_of_p == i)[0]
        cols = col_of[pi]
        xT = np.zeros((P128, npad), np.float32)
        xT[:, cols] = np.asarray(x)[pi].T
        im["xT"] = xT
        in_maps.append(im)
        assembly.append((pi, cols))

    return in_maps, assembly, nb, npad, c_max, [int(u) for u in u_max], G, spb


# ---------------------------------------------------------------------------
# Device program
# ---------------------------------------------------------------------------

def _build_nc(nb, npad, c_max, u_max, G, spb, repeat=1):
    """repeat>1 wraps the whole block loop in a HW For_i — used only by the
    benchmark to amortize the ~78ms axon dispatch RTT over R executions."""
    nch = nb * c_max
    nbg = (nb + G - 1) // G
    epb = c_max * P128

    nc = bass.Bass("TRN2", target_bir_lowering=False, debug=False)
    xT_t = nc.dram_tensor("xT", [P128, npad], F32, kind="ExternalInput")
    hc_ts = [
        nc.dram_tensor(f"hc{g}", [u_max[g], 256], F32, kind="ExternalInput")
        for g in range(G)
    ]
    slots_t = nc.dram_tensor("slots", [P128, nch], F32, kind="ExternalInput")
    idx_t = nc.dram_tensor("idx16", [P128, nb * spb], I16, kind="ExternalInput")
    wf_t = nc.dram_tensor("W_f", [128, 128], F32, kind="ExternalInput")
    uf_t = nc.dram_tensor("U_f", [128, 128], F32, kind="ExternalInput")
    wio_t = nc.dram_tensor("W_iou", [128, 384], F32, kind="ExternalInput")
    uio_t = nc.dram_tensor("U_iou", [128, 384], F32, kind="ExternalInput")
    bf_t = nc.dram_tensor("b_f", [128, 1], F32, kind="ExternalInput")
    bio_t = nc.dram_tensor("b_iou", [384, 1], F32, kind="ExternalInput")
    iota_t = nc.dram_tensor("iota", [128, S], F32, kind="ExternalInput")
    out_t = nc.dram_tensor("outT", [256, npad], F32, kind="ExternalOutput")

    # Emit the library load BEFORE the TileContext so it sits at the head of
    # the basic block, ahead of every Tile-scheduled Pool instruction — the
    # scheduler has no data dep to order it before the dma_gathers otherwise.
    nc.gpsimd.load_library(_mlp_library)  # dma_gather Q7 ucode

    with tile.TileContext(nc) as tc:
        with (
            tc.tile_pool(name="const", bufs=1) as cpool,
            tc.tile_pool(name="vpool", bufs=3) as vpool,
            tc.tile_pool(name="ppool", bufs=c_max + 2) as ppool,
            tc.tile_pool(name="xpool", bufs=3) as xpool,
            tc.tile_pool(name="hpool", bufs=2) as hpool,
            tc.tile_pool(name="fpool", bufs=2) as fpool,
            tc.tile_pool(name="gpool", bufs=2) as gpool,
            tc.tile_pool(name="spool", bufs=2) as spool,   # i sig
            tc.tile_pool(name="opool", bufs=2) as opool,   # o sig
            tc.tile_pool(name="upool", bufs=2) as upool,   # u tanh
            tc.tile_pool(name="npool", bufs=3) as npool,   # c_new
            tc.tile_pool(name="tpool", bufs=2) as tpool,   # tanh(c_new)
            tc.tile_pool(name="ypool", bufs=3) as ypool,   # h_new
            tc.tile_pool(name="psA", bufs=2, space="PSUM") as psA,
            tc.tile_pool(name="psB", bufs=2, space="PSUM") as psB,
            tc.tile_pool(name="psF", bufs=1, space="PSUM") as psF,
            tc.tile_pool(name="psIO", bufs=3, space="PSUM") as psIO,
        ):
            wf_sb = cpool.tile([128, 128], F32)
            nc.sync.dma_start(out=wf_sb[:], in_=wf_t[:, :])
            uf_sb = cpool.tile([128, 128], F32)
            nc.sync.dma_start(out=uf_sb[:], in_=uf_t[:, :])
            wio_sb = cpool.tile([128, 384], F32)
            nc.sync.dma_start(out=wio_sb[:], in_=wio_t[:, :])
            uio_sb = cpool.tile([128, 384], F32)
            nc.sync.dma_start(out=uio_sb[:], in_=uio_t[:, :])
            slots_sb = cpool.tile([P128, nch], F32)
            nc.sync.dma_start(out=slots_sb[:], in_=slots_t[:, :])
            idx_sb = cpool.tile([P128, nb * spb], I16)
            nc.sync.dma_start(out=idx_sb[:], in_=idx_t[:, :])
            bf_col = cpool.tile([128, 1], F32)
            nc.sync.dma_start(out=bf_col[:], in_=bf_t[:, :])
            bio_cols = []
            for t in range(3):
                col = cpool.tile([128, 1], F32, tag=f"bio_col{t}")
                nc.sync.dma_start(out=col[:], in_=bio_t[t * 128:(t + 1) * 128, :])
                bio_cols.append(col)
            # iota comes from the host: keeps GPSIMD on the mlp library only
            # (InstIota lives in the standard library; no reloads needed).
            iota_sb = cpool.tile([128, S], F32)
            nc.sync.dma_start(out=iota_sb[:], in_=iota_t[:, :])
            epb_reg = nc.gpsimd.to_reg(epb)  # shared num_idxs register

            from contextlib import ExitStack as _ES
            _loop_ctx = _ES()
            if repeat > 1:
                _loop_ctx.enter_context(tc.For_i(0, repeat, 1))
            for b in range(nb):
                g = b // nbg
                xT_sb = xpool.tile([128, S], F32)
                nc.sync.dma_start(out=xT_sb[:], in_=xT_t[:, b * S:(b + 1) * S])

                # one batched gather for the whole block's padded edges
                Vall = vpool.tile([128, c_max * 256], F32)
                nc.gpsimd.dma_gather(
                    out_ap=Vall[:].rearrange("p (c e) -> p c e", e=256),
                    in_ap=hc_ts[g][:, :],
                    idxs_ap=idx_sb[:, b * spb:(b + 1) * spb],
                    num_idxs=epb,
                    num_idxs_reg=epb_reg,
                    elem_size=256,
                )

                ps_hT = psA.tile([128, S], F32, space="PSUM")
                ps_cT = psB.tile([128, S], F32, space="PSUM")
                last = c_max - 1
                Ps = []
                for k in range(c_max):
                    j = b * c_max + k
                    P = ppool.tile([128, S], F32)
                    nc.vector.tensor_scalar(
                        out=P[:], in0=iota_sb[:], scalar1=slots_sb[:, j:j + 1],
                        scalar2=None, op0=ALU.is_equal,
                    )
                    Ps.append(P)
                # NB: matmuls of one PSUM accumulation group must be emitted
                # consecutively (interleaved groups on one bank miscompute).
                for k in range(c_max):
                    nc.tensor.matmul(
                        out=ps_hT[:], lhsT=Vall[:, k * 256:k * 256 + 128], rhs=Ps[k][:],
                        start=(k == 0), stop=(k == last),
                    )
                for k in range(c_max):
                    nc.tensor.matmul(
                        out=ps_cT[:], lhsT=Vall[:, k * 256 + 128:(k + 1) * 256], rhs=Ps[k][:],
                        start=(k == 0), stop=(k == last),
                    )

                hsumT_sb = hpool.tile([128, S], F32)
                nc.vector.tensor_copy(out=hsumT_sb[:], in_=ps_hT[:])

                ps_fT = psF.tile([128, S], F32, space="PSUM")
                nc.tensor.matmul(out=ps_fT[:], lhsT=wf_sb[:], rhs=xT_sb[:],
                                 start=True, stop=False)
                nc.tensor.matmul(out=ps_fT[:], lhsT=uf_sb[:], rhs=hsumT_sb[:],
                                 start=False, stop=True)
                fsig = fpool.tile([128, S], F32)
                nc.scalar.activation(out=fsig[:], in_=ps_fT[:], func=AF.Sigmoid,
                                     bias=bf_col[:])
                caggT = gpool.tile([128, S], F32)
                nc.vector.tensor_tensor(out=caggT[:], in0=fsig[:], in1=ps_cT[:],
                                        op=ALU.mult)

                iou_sb = []
                for t, func in ((0, AF.Sigmoid), (1, AF.Sigmoid), (2, AF.Tanh)):
                    ps = psIO.tile([128, S], F32, space="PSUM")
                    nc.tensor.matmul(out=ps[:], lhsT=wio_sb[:, t * 128:(t + 1) * 128],
                                     rhs=xT_sb[:], start=True, stop=False)
                    nc.tensor.matmul(out=ps[:], lhsT=uio_sb[:, t * 128:(t + 1) * 128],
                                     rhs=hsumT_sb[:], start=False, stop=True)
                    dest = (spool, opool, upool)[t].tile([128, S], F32)
                    nc.scalar.activation(out=dest[:], in_=ps[:], func=func,
                                         bias=bio_cols[t][:])
                    iou_sb.append(dest)
                isig, osig, utanh = iou_sb

                cnew = npool.tile([128, S], F32)
                nc.vector.tensor_tensor(out=cnew[:], in0=isig[:], in1=utanh[:],
                                        op=ALU.mult)
                nc.vector.tensor_tensor(out=cnew[:], in0=cnew[:], in1=caggT[:],
                                        op=ALU.add)
                tanhc = tpool.tile([128, S], F32)
                nc.scalar.activation(out=tanhc[:], in_=cnew[:], func=AF.Tanh)
                hnew = ypool.tile([128, S], F32)
                nc.vector.tensor_tensor(out=hnew[:], in0=osig[:], in1=tanhc[:],
                                        op=ALU.mult)
                nc.sync.dma_start(out=out_t[0:128, b * S:(b + 1) * S], in_=hnew[:])
                nc.sync.dma_start(out=out_t[128:256, b * S:(b + 1) * S], in_=cnew[:])
            _loop_ctx.close()

    _split_multi_waits(nc)
    lower_extended_insts(nc)  # populate .instr for InstPseudoReloadLibraryIndex
    return nc


LAST_EXEC_TIME_NS = None
LAST_RESULTS = None
_LAST_RUN = None  # (nc, in_maps) for benchmarking


def _make_runner(nc, in_maps, warmup=2):
    """Compile + stage a NEFF executor with device-resident inputs; returns a
    zero-arg callable measuring one blocking execute (wall seconds)."""
    import jax
    from jax.sharding import Mesh, PartitionSpec, NamedSharding
    try:
        from jax.experimental.shard_map import shard_map
    except ImportError:
        from jax.shard_map import shard_map
    from concourse import bass2jax

    bass2jax.install_neuronx_cc_hook()
    n_cores = len(in_maps)

    partition_name = nc.partition_id_tensor.name if nc.partition_id_tensor else None
    in_names, out_names, out_avals, zero_outs = [], [], [], []
    for alloc in nc.m.functions[0].allocations:
        if not isinstance(alloc, mybir.MemoryLocationSet):
            continue
        name = alloc.memorylocations[0].name
        if alloc.kind == "ExternalInput":
            if name != partition_name:
                in_names.append(name)
        elif alloc.kind == "ExternalOutput":
            shape = tuple(alloc.tensor_shape)
            dtype = mybir.dt.np(alloc.dtype)
            out_names.append(name)
            out_avals.append(jax.core.ShapedArray(shape, dtype))
            zero_outs.append(np.zeros(shape, dtype))
    n_params = len(in_names)
    all_names = in_names + out_names
    if partition_name is not None:
        all_names = all_names + [partition_name]

    def _body(*args):
        operands = list(args)
        if partition_name is not None:
            operands.append(bass2jax.partition_id_tensor())
        outs = bass2jax._bass_exec_p.bind(
            *operands,
            out_avals=tuple(out_avals),
            in_names=tuple(all_names),
            out_names=tuple(out_names),
            lowering_input_output_aliases=(),
            sim_require_finite=True,
            sim_require_nnan=True,
            nc=nc,
        )
        return tuple(outs)

    devices = jax.devices()[:n_cores]
    mesh = Mesh(np.asarray(devices), ("core",))
    spec = PartitionSpec("core")
    fn = jax.jit(
        shard_map(
            _body, mesh=mesh,
            in_specs=(spec,) * (n_params + len(out_names)),
            out_specs=(spec,) * len(out_names),
            check_rep=False,
        ),
        keep_unused=True,
    )
    sh = NamedSharding(mesh, spec)
    args = [
        jax.device_put(
            np.concatenate([np.asarray(in_maps[c][nm]) for c in range(n_cores)], axis=0), sh
        )
        for nm in in_names
    ] + [
        jax.device_put(np.concatenate([z] * n_cores, axis=0), sh) for z in zero_outs
    ]

    for _ in range(warmup):
        out = fn(*args)
    jax.block_until_ready(out)

    def call():
        t0 = time.perf_counter()
        out = fn(*args)
        jax.block_until_ready(out)
        return time.perf_counter() - t0

    return call


_LAST_BUILD_ARGS = None


def benchmark_last(iters=24, reps=8):
    """Device-time estimate that defeats the ~60-80ms axon dispatch RTT (and
    its drift): build a variant of the same kernel whose block loop runs
    `reps` times inside a HW For_i (the kernel is idempotent), INTERLEAVE
    R=1 / R=reps calls so network drift cancels, then
      device_ns = (min_call(R=reps) - min_call(R=1)) / (reps - 1)."""
    global LAST_EXEC_TIME_NS
    assert _LAST_RUN is not None, "call kernel() first"
    nc1, in_maps = _LAST_RUN
    nb, npad, c_max, u_max, G, spb = _LAST_BUILD_ARGS
    nc_r = _build_nc(nb, npad, c_max, u_max, G, spb, repeat=reps)
    call1 = _make_runner(nc1, in_maps)
    callr = _make_runner(nc_r, in_maps)
    t1 = tr = float("inf")
    for _ in range(iters):
        t1 = min(t1, call1())
        tr = min(tr, callr())
    t1, tr = int(t1 * 1e9), int(tr * 1e9)
    dev = int((tr - t1) / (reps - 1))
    print(f"  [bench] min per-call: R=1: {t1} ns, R={reps}: {tr} ns"
          f" -> device ~{dev} ns/exec")
    LAST_EXEC_TIME_NS = dev if dev > 0 else t1
    return LAST_EXEC_TIME_NS


def kernel(x, h, c, child_idx, parent_idx, W_f, U_f, b_f, W_iou, U_iou, b_iou,
           trace=False, trace_cores=None):
    global LAST_EXEC_TIME_NS, LAST_RESULTS, _LAST_RUN, _LAST_BUILD_ARGS
    x = np.asarray(x, np.float32)
    N = x.shape[0]
    in_maps, assembly, nb, npad, c_max, u_max, G, spb = _prep(
        x, h, c, child_idx, parent_idx)

    _LAST_BUILD_ARGS = (nb, npad, c_max, u_max, G, spb)
    nc = _build_nc(nb, npad, c_max, u_max, G, spb)
    for im in in_maps:
        im["W_f"] = np.asarray(W_f, np.float32)
        im["U_f"] = np.asarray(U_f, np.float32)
        im["W_iou"] = np.asarray(W_iou, np.float32)
        im["U_iou"] = np.asarray(U_iou, np.float32)
        im["b_f"] = np.asarray(b_f, np.float32).reshape(128, 1)
        im["b_iou"] = np.asarray(b_iou, np.float32).reshape(384, 1)
        im["iota"] = np.broadcast_to(
            np.arange(S, dtype=np.float32)[None, :], (P128, S)).copy()

    kwargs = {}
    if trace:
        kwargs["trace"] = True
        if trace_cores is not None:
            kwargs["trace_cores"] = trace_cores

    for attempt in range(3):
        res = run_bass_kernel_spmd(nc, in_maps, core_ids=list(range(CORES)), **kwargs)
        LAST_EXEC_TIME_NS = res.exec_time_ns
        LAST_RESULTS = res
        _LAST_RUN = (nc, in_maps)
        out = np.empty((N, 256), np.float32)
        for i, (pi, cols) in enumerate(assembly):
            out[pi] = res.results[i]["outT"].T[cols]
        err = _sample_check(out, x, np.asarray(h), np.asarray(c),
                            np.asarray(child_idx), np.asarray(parent_idx),
                            W_f, U_f, b_f, W_iou, U_iou, b_iou)
        if err < 1e-3:
            break
        print(f"  [kernel] sample self-check failed (rel {err:.3e}); "
              f"retrying (device flake?)")
    return out


def _sample_check(out, x, h, c, child_idx, parent_idx,
                  W_f, U_f, b_f, W_iou, U_iou, b_iou, k=64):
    """Spot-check k random nodes against a numpy reference; catches silent
    device flakes (observed once: garbage output with no runtime error)."""
    rng = np.random.default_rng(0)
    nodes = rng.choice(x.shape[0], size=min(k, x.shape[0]), replace=False)
    sel = {int(n): i for i, n in enumerate(nodes)}
    hs = np.zeros((len(nodes), 128), np.float64)
    cs = np.zeros((len(nodes), 128), np.float64)
    m = np.isin(parent_idx, nodes)
    for p, ch in zip(parent_idx[m], child_idx[m]):
        i = sel[int(p)]
        hs[i] += h[ch]
        cs[i] += c[ch]
    xs = x[nodes].astype(np.float64)

    def sig(v):
        return 1.0 / (1.0 + np.exp(-v))

    f = sig(xs @ W_f + hs @ U_f + np.asarray(b_f))
    iou = xs @ W_iou + hs @ U_iou + np.asarray(b_iou)
    i_, o, u = np.split(iou, 3, axis=1)
    cn = sig(i_) * np.tanh(u) + f * cs
    hn = sig(o) * np.tanh(cn)
    exp = np.concatenate([hn, cn], axis=1)
    return float(np.abs(out[nodes] - exp).max() / max(1e-9, np.abs(exp).max()))



# revision 4
# speedup vs baseline: 9.9937x; 9.9937x over previous
"""ChildSum TreeLSTM cell on 8 Trainium2 NeuronCores (Bass/Tile).

Strategy (graph-parallel, per the sharding hint):
  - Partition nodes (parents) into 8 contiguous ranges of N/8; each core owns
    the segment-sum + cell update for its parents.
  - Host does INDEX prep only: near-LPT assignment of parents to 512-parent
    blocks (bounds block edge count near the mean), within-block assignment of
    parents to four 128-slot windows (balances edges per window), per-core
    compacted child tables (for int16 dma_gather indices), one-hot P matrices
    for the segment-sum (index prep: P encodes edge->slot, values 0/1),
    pre-transposed fp16 x.
  - Device does all FLOP/memory work in fp16 (fp32 PSUM accumulation):
    batched dma_gather of child h||c rows, slot-windowed segment-sum on the
    PE (4 chunks of 128 edges -> 128-slot windows plus one 512-wide catch-all
    chunk for window overflow; ONE PSUM accumulation group per bank), dense
    LSTM matmuls, sigmoid/tanh on ACT, elementwise on DVE, fp16 output store.

Everything on device is computed in TRANSPOSED orientation [feature, node]:
  V[e, 0:128]=h_child, V[e,128:256]=c_child    (5*128 gathered edges/block)
  h_sumT[h, s] += V_h^T P_k                    (PE, chunk k; P from host)
  fT/iT/oT/uT[h, n] = W^T xT + U^T h_sumT      (PE; W/U natural layout as lhsT)
  biases ride the ACT `bias` operand (per-partition = per-feature).
Output is written transposed fp16 [256, npad] and de-transposed on host.

Engine placement notes (trn2):
  - No DVE tensor_scalar/cast ops in the loop: 2-port DVE perf-mode ops lock
    the SBUF port pair shared with GPSIMD and starve SWDGE descriptor
    generation for the gather (measured 7.3ms -> this design avoids it).
  - Plain DMAs ride HWDGE (nc.sync, SP queue), gather rides SWDGE (Pool).
  - DVE keeps only 1-port ops: tensor_tensor and PSUM-source tensor_copy.
"""

import os
import sys
import time

for _p in ("/opt/trn_rl_repo", "/root/.axon_site/_ro/trn_rl_repo"):
    if os.path.isdir(_p) and _p not in sys.path:
        sys.path.insert(0, _p)

import numpy as np

import concourse.bass as bass
import concourse.tile as tile
from concourse import mybir
from concourse.bass_utils import run_bass_kernel_spmd
from concourse.library_config import mlp as _mlp_library
from concourse.library_overlay import lower_extended_insts
from concourse.vector_clock import ScopedClock

CORES = 8
S = 512          # parents per block (= PSUM bank free dim in fp32)
P128 = 128
NW = 4           # slot windows per block (128 slots each)
CH = NW + 1      # gather chunks per block: 4 windowed + 1 catch-all
EPB = CH * P128  # padded edges per block (640)
SPB = EPB // 16  # int16 idx columns per block (40)
PCOLS = NW * P128 + S   # P matrix cols per block: 4x128 windows + 512 catch-all
MAX_IDX16 = 32000  # max rows per gather table (int16 indices)

F32 = mybir.dt.float32
F16 = mybir.dt.float16
I16 = mybir.dt.int16
AF = mybir.ActivationFunctionType
ALU = mybir.AluOpType

# ---------------------------------------------------------------------------
# Workarounds: the walrus build in this container accepts at most ONE sync
# wait per instruction. (a) chunk the Tile tail-drain waits onto nops;
# (b) post-pass that hoists extra waits of any instruction onto preceding
# NoOps on the same engine.
# ---------------------------------------------------------------------------

def _drain_and_barrier_chunked(self, tick_clock, wait_clock):
    probe = self.nc.sync.nop()
    wait_clock.add_sem_waits(probe.ins, ScopedClock({None: tick_clock.global_clock}))
    si = probe.ins.sync_info
    waits = list(si.on_wait) if si is not None else []
    if si is not None:
        probe.ins.sync_info = mybir.SyncInfo(on_wait=waits[:1], on_update=list(si.on_update))
    for i in range(1, len(waits)):
        nop = self.nc.sync.nop()
        nop.ins.sync_info = mybir.SyncInfo(on_wait=waits[i:i + 1], on_update=[])
    self.nc.sync.drain()
    self.nc.all_engine_barrier()
    popped = self.nc._tile_sem_poison_stack.pop()
    assert popped is self._sem_poison
    self.nc.clear_and_free_semaphores(list(self.sems.allocated().values()))
    self.nc.all_engine_barrier()


tile.TileContext._drain_and_barrier = _drain_and_barrier_chunked

_WSPLIT_CTR = [0]


def _split_multi_waits(nc):
    n_split = 0
    for f in nc.m.functions:
        for bb in f.blocks:
            insts = list(bb.instructions)
            if not any(
                i.sync_info is not None and i.sync_info.on_wait and len(i.sync_info.on_wait) > 1
                for i in insts
            ):
                continue
            new = []
            for inst in insts:
                si = inst.sync_info
                if si is not None and si.on_wait and len(si.on_wait) > 1:
                    waits = list(si.on_wait)
                    n_split += 1
                    for w in waits[:-1]:
                        _WSPLIT_CTR[0] += 1
                        new.append(
                            mybir.InstNoOp(
                                name=f"I-wsplit-{_WSPLIT_CTR[0]}",
                                engine=inst.engine,
                                debug=inst.debug,
                                ins=[],
                                outs=[],
                                sync_info=mybir.SyncInfo(on_wait=[w], on_update=[]),
                            )
                        )
                    inst.sync_info = mybir.SyncInfo(
                        on_wait=[waits[-1]], on_update=list(si.on_update)
                    )
                new.append(inst)
            bb.instructions = new
    return n_split


# ---------------------------------------------------------------------------
# Host-side index prep
# ---------------------------------------------------------------------------

def _prep(x, h, c, child_idx, parent_idx):
    N = x.shape[0]
    npc = (N + CORES - 1) // CORES            # parents per core
    nb = (npc + S - 1) // S                   # blocks per core
    npad = nb * S
    nbt = CORES * nb                          # total blocks

    parent = np.asarray(parent_idx).astype(np.int64)
    child = np.asarray(child_idx).astype(np.int64)

    # ---- near-LPT parent -> block assignment (bounds every block's edge
    # count near the mean; the relabeling is free: xT columns and output rows
    # are permuted on the host anyway).
    deg = np.bincount(parent, minlength=N)
    loads = np.zeros(nbt, np.int64)
    pcount = np.zeros(nbt, np.int64)
    gblock = np.empty(N, np.int64)
    for d in range(int(deg.max()), 0, -1):
        members = np.nonzero(deg == d)[0]
        if len(members) == 0:
            continue
        border = np.argsort(loads, kind="stable")
        k = len(members)
        slots_assign = np.tile(border, -(-k // nbt))[:k]
        gblock[members] = slots_assign
        loads += np.bincount(slots_assign, minlength=nbt) * d
        pcount += np.bincount(slots_assign, minlength=nbt)
    # zero-degree parents fill remaining slot capacity exactly
    d0 = np.nonzero(deg == 0)[0]
    cap = S - pcount
    fill = np.repeat(np.arange(nbt), cap)[: len(d0)]
    gblock[d0] = fill
    pcount += np.bincount(fill, minlength=nbt)
    assert pcount.max() <= S, pcount.max()
    assert loads.max() <= NW * P128 + P128, loads.max()

    # ---- within-block slot assignment: snake-deal parents (sorted by degree
    # desc) across the NW windows so every window's edge count is near the
    # mean; window overflow beyond 128 edges goes to the catch-all chunk.
    order_p = np.lexsort((np.arange(N), -deg, gblock))  # block asc, deg desc
    counts = np.bincount(gblock, minlength=nbt)
    starts = np.zeros(nbt + 1, np.int64)
    starts[1:] = np.cumsum(counts)
    rank_in_block = np.arange(N) - starts[gblock[order_p]]
    # snake pattern 0,1,..,NW-1,NW-1,..,1,0 repeating
    snake = np.concatenate([np.arange(NW), np.arange(NW)[::-1]])
    win_of_rank = snake[rank_in_block % (2 * NW)]
    # position within window = count of earlier ranks in same window
    pos_in_win = rank_in_block // (2 * NW) * 2
    first_pass = rank_in_block % (2 * NW) < NW
    pos_in_win = pos_in_win + (~first_pass).astype(np.int64)
    slot_of = np.empty(N, np.int64)
    slot_of[order_p] = win_of_rank * P128 + pos_in_win
    win_of = np.empty(N, np.int64)
    win_of[order_p] = win_of_rank

    # ---- edges sorted by (block, window, child): locality + chunking
    eblock = gblock[parent]
    ewin = win_of[parent]
    eorder = np.lexsort((child, ewin, eblock))
    se_block = eblock[eorder]
    se_win = ewin[eorder]
    se_child = child[eorder]
    se_slot = slot_of[parent][eorder]
    ebw_key = se_block * NW + se_win
    ebw_counts = np.bincount(ebw_key, minlength=nbt * NW)
    ebw_starts = np.zeros(nbt * NW + 1, np.int64)
    ebw_starts[1:] = np.cumsum(ebw_counts)

    hc = np.concatenate([np.asarray(h), np.asarray(c)], axis=1).astype(np.float16)

    # per-core arrays
    core_child = []     # [nb*EPB] gather child (global id), padding=0
    core_Ps = []        # [128, nb*PCOLS] fp16 one-hot
    max_ca = 0
    for i in range(CORES):
        gidx = np.zeros(nb * EPB, np.int64)
        P = np.zeros((P128, nb * PCOLS), np.float16)
        for b in range(nb):
            gb = i * nb + b
            ca_ch = []
            ca_sl = []
            for w in range(NW):
                e0, e1 = ebw_starts[gb * NW + w], ebw_starts[gb * NW + w + 1]
                mm = e1 - e0
                take = min(mm, P128)
                ch = se_child[e0:e0 + take]
                sl = se_slot[e0:e0 + take] - w * P128
                off = b * EPB + w * P128
                gidx[off:off + take] = ch
                # edge linear index off+j -> partition (off+j) % 128 , chunk w
                P[np.arange(take), b * PCOLS + w * P128 + sl] = 1.0
                if mm > take:
                    ca_ch.extend(se_child[e0 + take:e1])
                    ca_sl.extend(se_slot[e0 + take:e1])
            nca = len(ca_ch)
            assert nca <= P128, f"catch-all overflow {nca} in core {i} block {b}"
            max_ca = max(max_ca, nca)
            if nca:
                off = b * EPB + NW * P128
                gidx[off:off + nca] = ca_ch
                P[np.arange(nca), b * PCOLS + NW * P128 + np.asarray(ca_sl)] = 1.0
        core_child.append(gidx)
        core_Ps.append(P)

    # ---- split blocks into G groups so every per-core group table fits int16
    G = 1
    while True:
        nbg = (nb + G - 1) // G
        ok = True
        groups = []  # per core: list of (b0, b1, uniq, remapped idx)
        u_max = [0] * G
        for i in range(CORES):
            gi = []
            for g in range(G):
                b0, b1 = g * nbg, min((g + 1) * nbg, nb)
                seg = slice(b0 * EPB, b1 * EPB)
                ch = core_child[i][seg]
                uniq = np.unique(ch)
                if len(uniq) > MAX_IDX16:
                    ok = False
                    break
                rem = np.searchsorted(uniq, ch)
                u_max[g] = max(u_max[g], len(uniq))
                gi.append((b0, b1, uniq, rem))
            if not ok:
                break
            groups.append(gi)
        if ok:
            break
        G += 1

    core_of_p = gblock // nb
    col_of = (gblock % nb) * S + slot_of
    in_maps = []
    assembly = []
    xx = np.asarray(x)
    for i in range(CORES):
        im = {}
        idx16 = np.zeros((P128, nb * SPB), np.int16)
        for g in range(G):
            b0, b1, uniq, rem = groups[i][g]
            tbl = np.zeros((u_max[g], 256), np.float16)
            tbl[: len(uniq)] = hc[uniq]
            im[f"hc{g}"] = tbl
            for b in range(b0, b1):
                e = rem[(b - b0) * EPB:(b - b0 + 1) * EPB].astype(np.int16)
                # dma_gather order: edge i = col*16 + partition(p<16); replicate x8
                blk = e.reshape(SPB, 16).T          # [16, SPB]
                idx16[:, b * SPB:(b + 1) * SPB] = np.tile(blk, (8, 1))
        im["idx16"] = idx16
        im["P"] = np.ascontiguousarray(core_Ps[i])
        pi = np.nonzero(core_of_p == i)[0]
        cols = col_of[pi]
        xT = np.zeros((P128, npad), np.float16)
        xT[:, cols] = xx[pi].T.astype(np.float16)
        im["xT"] = xT
        in_maps.append(im)
        assembly.append((pi, cols))

    return in_maps, assembly, nb, npad, [int(u) for u in u_max], G


# ---------------------------------------------------------------------------
# Device program
# ---------------------------------------------------------------------------

def _build_nc(nb, npad, u_max, G, repeat=1):
    """repeat>1 wraps the whole block loop in a HW For_i — used only by the
    benchmark to amortize the ~78ms axon dispatch RTT over R executions."""
    nbg = (nb + G - 1) // G

    nc = bass.Bass("TRN2", target_bir_lowering=False, debug=False)
    xT_t = nc.dram_tensor("xT", [P128, npad], F16, kind="ExternalInput")
    hc_ts = [
        nc.dram_tensor(f"hc{g}", [u_max[g], 256], F16, kind="ExternalInput")
        for g in range(G)
    ]
    P_t = nc.dram_tensor("P", [P128, nb * PCOLS], F16, kind="ExternalInput")
    idx_t = nc.dram_tensor("idx16", [P128, nb * SPB], I16, kind="ExternalInput")
    wf_t = nc.dram_tensor("W_f", [128, 128], F16, kind="ExternalInput")
    uf_t = nc.dram_tensor("U_f", [128, 128], F16, kind="ExternalInput")
    wio_t = nc.dram_tensor("W_iou", [128, 384], F16, kind="ExternalInput")
    uio_t = nc.dram_tensor("U_iou", [128, 384], F16, kind="ExternalInput")
    bf_t = nc.dram_tensor("b_f", [128, 1], F32, kind="ExternalInput")
    bio_t = nc.dram_tensor("b_iou", [384, 1], F32, kind="ExternalInput")
    out_t = nc.dram_tensor("outT", [256, npad], F16, kind="ExternalOutput")

    # Emit the library load BEFORE the TileContext so it sits at the head of
    # the basic block, ahead of every Tile-scheduled Pool instruction — the
    # scheduler has no data dep to order it before the dma_gathers otherwise.
    nc.gpsimd.load_library(_mlp_library)  # dma_gather Q7 ucode

    with tile.TileContext(nc) as tc:
        with (
            tc.tile_pool(name="const", bufs=1) as cpool,
            tc.tile_pool(name="vpool", bufs=3) as vpool,
            tc.tile_pool(name="Ppool", bufs=3) as Ppool,
            tc.tile_pool(name="xpool", bufs=3) as xpool,
            tc.tile_pool(name="hpool", bufs=2) as hpool,
            tc.tile_pool(name="fpool", bufs=2) as fpool,
            tc.tile_pool(name="gpool", bufs=2) as gpool,
            tc.tile_pool(name="spool", bufs=2) as spool,   # i sig
            tc.tile_pool(name="opool", bufs=2) as opool,   # o sig
            tc.tile_pool(name="upool", bufs=2) as upool,   # u tanh
            tc.tile_pool(name="npool", bufs=3) as npool,   # c_new
            tc.tile_pool(name="tpool", bufs=2) as tpool,   # tanh(c_new)
            tc.tile_pool(name="ypool", bufs=3) as ypool,   # h_new
            tc.tile_pool(name="psA", bufs=2, space="PSUM") as psA,
            tc.tile_pool(name="psB", bufs=2, space="PSUM") as psB,
            tc.tile_pool(name="psF", bufs=1, space="PSUM") as psF,
            tc.tile_pool(name="psIO", bufs=3, space="PSUM") as psIO,
        ):
            wf_sb = cpool.tile([128, 128], F16)
            nc.sync.dma_start(out=wf_sb[:], in_=wf_t[:, :])
            uf_sb = cpool.tile([128, 128], F16)
            nc.sync.dma_start(out=uf_sb[:], in_=uf_t[:, :])
            wio_sb = cpool.tile([128, 384], F16)
            nc.sync.dma_start(out=wio_sb[:], in_=wio_t[:, :])
            uio_sb = cpool.tile([128, 384], F16)
            nc.sync.dma_start(out=uio_sb[:], in_=uio_t[:, :])
            idx_sb = cpool.tile([P128, nb * SPB], I16)
            nc.sync.dma_start(out=idx_sb[:], in_=idx_t[:, :])
            bf_col = cpool.tile([128, 1], F32)
            nc.sync.dma_start(out=bf_col[:], in_=bf_t[:, :])
            bio_cols = []
            for t in range(3):
                col = cpool.tile([128, 1], F32, tag=f"bio_col{t}")
                nc.sync.dma_start(out=col[:], in_=bio_t[t * 128:(t + 1) * 128, :])
                bio_cols.append(col)
            epb_reg = nc.gpsimd.to_reg(EPB)  # shared num_idxs register

            from contextlib import ExitStack as _ES
            _loop_ctx = _ES()
            if repeat > 1:
                _loop_ctx.enter_context(tc.For_i(0, repeat, 1))
            for b in range(nb):
                g = b // nbg
                xT_sb = xpool.tile([128, S], F16)
                nc.sync.dma_start(out=xT_sb[:], in_=xT_t[:, b * S:(b + 1) * S])
                P_sb = Ppool.tile([128, PCOLS], F16)
                nc.sync.dma_start(out=P_sb[:], in_=P_t[:, b * PCOLS:(b + 1) * PCOLS])

                # one batched gather for the whole block's padded edges
                Vall = vpool.tile([128, CH * 256], F16)
                nc.gpsimd.dma_gather(
                    out_ap=Vall[:].rearrange("p (c e) -> p c e", e=256),
                    in_ap=hc_ts[g][:, :],
                    idxs_ap=idx_sb[:, b * SPB:(b + 1) * SPB],
                    num_idxs=EPB,
                    num_idxs_reg=epb_reg,
                    elem_size=256,
                )

                # slot-windowed segment sum. Order-independent formulation:
                # the catch-all chunk goes FIRST with start=True — it clears
                # the whole 2KB zero region AND writes all 512 columns (sets
                # every has_written bit), so each window matmul is a pure
                # accumulate into its 128-col region. The windows' mutual
                # order doesn't matter, and each has a WAW dep on the
                # catch-all (overlapping region), so Tile can't hoist them
                # above it. stop on the last-emitted window is sim-only
                # bookkeeping (no HW effect); skip_group_check because the
                # scheduler may run windows in any order.
                ps_hT = psA.tile([128, S], F32, space="PSUM")
                ps_cT = psB.tile([128, S], F32, space="PSUM")
                nc.tensor.matmul(
                    out=ps_hT[:], lhsT=Vall[:, NW * 256:NW * 256 + 128],
                    rhs=P_sb[:, NW * P128:NW * P128 + S],
                    start=True, stop=False,
                )
                for k in range(NW):
                    nc.tensor.matmul(
                        out=ps_hT[:, k * P128:(k + 1) * P128],
                        lhsT=Vall[:, k * 256:k * 256 + 128],
                        rhs=P_sb[:, k * P128:(k + 1) * P128],
                        start=False, stop=(k == NW - 1),
                        skip_group_check=(k != NW - 1),
                    )
                nc.tensor.matmul(
                    out=ps_cT[:], lhsT=Vall[:, NW * 256 + 128:(NW + 1) * 256],
                    rhs=P_sb[:, NW * P128:NW * P128 + S],
                    start=True, stop=False,
                )
                for k in range(NW):
                    nc.tensor.matmul(
                        out=ps_cT[:, k * P128:(k + 1) * P128],
                        lhsT=Vall[:, k * 256 + 128:(k + 1) * 256],
                        rhs=P_sb[:, k * P128:(k + 1) * P128],
                        start=False, stop=(k == NW - 1),
                        skip_group_check=(k != NW - 1),
                    )

                hsumT_sb = hpool.tile([128, S], F16)
                nc.vector.tensor_copy(out=hsumT_sb[:], in_=ps_hT[:])

                ps_fT = psF.tile([128, S], F32, space="PSUM")
                nc.tensor.matmul(out=ps_fT[:], lhsT=wf_sb[:], rhs=xT_sb[:],
                                 start=True, stop=False)
                nc.tensor.matmul(out=ps_fT[:], lhsT=uf_sb[:], rhs=hsumT_sb[:],
                                 start=False, stop=True)
                fsig = fpool.tile([128, S], F16)
                nc.scalar.activation(out=fsig[:], in_=ps_fT[:], func=AF.Sigmoid,
                                     bias=bf_col[:])
                caggT = gpool.tile([128, S], F16)
                nc.vector.tensor_tensor(out=caggT[:], in0=fsig[:], in1=ps_cT[:],
                                        op=ALU.mult)

                iou_sb = []
                for t, func in ((0, AF.Sigmoid), (1, AF.Sigmoid), (2, AF.Tanh)):
                    ps = psIO.tile([128, S], F32, space="PSUM")
                    nc.tensor.matmul(out=ps[:], lhsT=wio_sb[:, t * 128:(t + 1) * 128],
                                     rhs=xT_sb[:], start=True, stop=False)
                    nc.tensor.matmul(out=ps[:], lhsT=uio_sb[:, t * 128:(t + 1) * 128],
                                     rhs=hsumT_sb[:], start=False, stop=True)
                    dest = (spool, opool, upool)[t].tile([128, S], F16)
                    nc.scalar.activation(out=dest[:], in_=ps[:], func=func,
                                         bias=bio_cols[t][:])
                    iou_sb.append(dest)
                isig, osig, utanh = iou_sb

                cnew = npool.tile([128, S], F16)
                nc.vector.tensor_tensor(out=cnew[:], in0=isig[:], in1=utanh[:],
                                        op=ALU.mult)
                nc.vector.tensor_tensor(out=cnew[:], in0=cnew[:], in1=caggT[:],
                                        op=ALU.add)
                tanhc = tpool.tile([128, S], F16)
                nc.scalar.activation(out=tanhc[:], in_=cnew[:], func=AF.Tanh)
                hnew = ypool.tile([128, S], F16)
                nc.vector.tensor_tensor(out=hnew[:], in0=osig[:], in1=tanhc[:],
                                        op=ALU.mult)
                nc.sync.dma_start(out=out_t[0:128, b * S:(b + 1) * S], in_=hnew[:])
                nc.sync.dma_start(out=out_t[128:256, b * S:(b + 1) * S], in_=cnew[:])
            _loop_ctx.close()

    _split_multi_waits(nc)
    lower_extended_insts(nc)  # populate .instr for InstPseudoReloadLibraryIndex
    return nc


LAST_EXEC_TIME_NS = None
LAST_RESULTS = None
_LAST_RUN = None  # (nc, in_maps) for benchmarking


def _make_runner(nc, in_maps, warmup=2):
    """Compile + stage a NEFF executor with device-resident inputs; returns a
    zero-arg callable measuring one blocking execute (wall seconds)."""
    import jax
    from jax.sharding import Mesh, PartitionSpec, NamedSharding
    try:
        from jax.experimental.shard_map import shard_map
    except ImportError:
        from jax.shard_map import shard_map
    from concourse import bass2jax

    bass2jax.install_neuronx_cc_hook()
    n_cores = len(in_maps)

    partition_name = nc.partition_id_tensor.name if nc.partition_id_tensor else None
    in_names, out_names, out_avals, zero_outs = [], [], [], []
    for alloc in nc.m.functions[0].allocations:
        if not isinstance(alloc, mybir.MemoryLocationSet):
            continue
        name = alloc.memorylocations[0].name
        if alloc.kind == "ExternalInput":
            if name != partition_name:
                in_names.append(name)
        elif alloc.kind == "ExternalOutput":
            shape = tuple(alloc.tensor_shape)
            dtype = mybir.dt.np(alloc.dtype)
            out_names.append(name)
            out_avals.append(jax.core.ShapedArray(shape, dtype))
            zero_outs.append(np.zeros(shape, dtype))
    n_params = len(in_names)
    all_names = in_names + out_names
    if partition_name is not None:
        all_names = all_names + [partition_name]

    def _body(*args):
        operands = list(args)
        if partition_name is not None:
            operands.append(bass2jax.partition_id_tensor())
        outs = bass2jax._bass_exec_p.bind(
            *operands,
            out_avals=tuple(out_avals),
            in_names=tuple(all_names),
            out_names=tuple(out_names),
            lowering_input_output_aliases=(),
            sim_require_finite=True,
            sim_require_nnan=True,
            nc=nc,
        )
        return tuple(outs)

    devices = jax.devices()[:n_cores]
    mesh = Mesh(np.asarray(devices), ("core",))
    spec = PartitionSpec("core")
    fn = jax.jit(
        shard_map(
            _body, mesh=mesh,
            in_specs=(spec,) * (n_params + len(out_names)),
            out_specs=(spec,) * len(out_names),
            check_rep=False,
        ),
        keep_unused=True,
    )
    sh = NamedSharding(mesh, spec)
    args = [
        jax.device_put(
            np.concatenate([np.asarray(in_maps[c][nm]) for c in range(n_cores)], axis=0), sh
        )
        for nm in in_names
    ] + [
        jax.device_put(np.concatenate([z] * n_cores, axis=0), sh) for z in zero_outs
    ]

    for _ in range(warmup):
        out = fn(*args)
    jax.block_until_ready(out)

    def call():
        t0 = time.perf_counter()
        out = fn(*args)
        jax.block_until_ready(out)
        return time.perf_counter() - t0

    return call


_LAST_BUILD_ARGS = None


def benchmark_last(iters=24, reps=8):
    """Device-time estimate that defeats the ~60-80ms axon dispatch RTT (and
    its drift): build a variant of the same kernel whose block loop runs
    `reps` times inside a HW For_i (the kernel is idempotent), INTERLEAVE
    R=1 / R=reps calls so network drift cancels, then
      device_ns = (min_call(R=reps) - min_call(R=1)) / (reps - 1)."""
    global LAST_EXEC_TIME_NS
    assert _LAST_RUN is not None, "call kernel() first"
    nc1, in_maps = _LAST_RUN
    nb, npad, u_max, G = _LAST_BUILD_ARGS
    nc_r = _build_nc(nb, npad, u_max, G, repeat=reps)
    call1 = _make_runner(nc1, in_maps)
    callr = _make_runner(nc_r, in_maps)
    t1 = tr = float("inf")
    for _ in range(iters):
        t1 = min(t1, call1())
        tr = min(tr, callr())
    t1, tr = int(t1 * 1e9), int(tr * 1e9)
    dev = int((tr - t1) / (reps - 1))
    print(f"  [bench] min per-call: R=1: {t1} ns, R={reps}: {tr} ns"
          f" -> device ~{dev} ns/exec")
    LAST_EXEC_TIME_NS = dev if dev > 0 else t1
    return LAST_EXEC_TIME_NS


def kernel(x, h, c, child_idx, parent_idx, W_f, U_f, b_f, W_iou, U_iou, b_iou,
           trace=False, trace_cores=None):
    global LAST_EXEC_TIME_NS, LAST_RESULTS, _LAST_RUN, _LAST_BUILD_ARGS
    x = np.asarray(x, np.float32)
    N = x.shape[0]
    in_maps, assembly, nb, npad, u_max, G = _prep(
        x, h, c, child_idx, parent_idx)

    _LAST_BUILD_ARGS = (nb, npad, u_max, G)
    nc = _build_nc(nb, npad, u_max, G)
    for im in in_maps:
        im["W_f"] = np.asarray(W_f, np.float16)
        im["U_f"] = np.asarray(U_f, np.float16)
        im["W_iou"] = np.asarray(W_iou, np.float16)
        im["U_iou"] = np.asarray(U_iou, np.float16)
        im["b_f"] = np.asarray(b_f, np.float32).reshape(128, 1)
        im["b_iou"] = np.asarray(b_iou, np.float32).reshape(384, 1)

    kwargs = {}
    if trace:
        kwargs["trace"] = True
        if trace_cores is not None:
            kwargs["trace_cores"] = trace_cores

    for attempt in range(3):
        res = run_bass_kernel_spmd(nc, in_maps, core_ids=list(range(CORES)), **kwargs)
        LAST_EXEC_TIME_NS = res.exec_time_ns
        LAST_RESULTS = res
        _LAST_RUN = (nc, in_maps)
        out = np.empty((N, 256), np.float32)
        for i, (pi, cols) in enumerate(assembly):
            out[pi] = res.results[i]["outT"].T[cols].astype(np.float32)
        err = _sample_check(out, x, np.asarray(h), np.asarray(c),
                            np.asarray(child_idx), np.asarray(parent_idx),
                            W_f, U_f, b_f, W_iou, U_iou, b_iou)
        if err < 1e-2:
            break
        print(f"  [kernel] sample self-check failed (rel {err:.3e}); "
              f"retrying (device flake?)")
    return out


def _sample_check(out, x, h, c, child_idx, parent_idx,
                  W_f, U_f, b_f, W_iou, U_iou, b_iou, k=64):
    """Spot-check k random nodes against a numpy reference; catches silent
    device flakes (observed once: garbage output with no runtime error)."""
    rng = np.random.default_rng(0)
    nodes = rng.choice(x.shape[0], size=min(k, x.shape[0]), replace=False)
    sel = {int(n): i for i, n in enumerate(nodes)}
    hs = np.zeros((len(nodes), 128), np.float64)
    cs = np.zeros((len(nodes), 128), np.float64)
    m = np.isin(parent_idx, nodes)
    for p, ch in zip(parent_idx[m], child_idx[m]):
        i = sel[int(p)]
        hs[i] += h[ch]
        cs[i] += c[ch]
    xs = x[nodes].astype(np.float64)

    def sig(v):
        return 1.0 / (1.0 + np.exp(-v))

    f = sig(xs @ W_f + hs @ U_f + np.asarray(b_f))
    iou = xs @ W_iou + hs @ U_iou + np.asarray(b_iou)
    i_, o, u = np.split(iou, 3, axis=1)
    cn = sig(i_) * np.tanh(u) + f * cs
    hn = sig(o) * np.tanh(cn)
    exp = np.concatenate([hn, cn], axis=1)
    return float(np.abs(out[nodes] - exp).max() / max(1e-9, np.abs(exp).max()))


# revision 6
# speedup vs baseline: 13.6564x; 1.3665x over previous
"""ChildSum TreeLSTM cell on 8 Trainium2 NeuronCores (Bass/Tile).

Strategy (graph-parallel, per the sharding hint):
  - Partition nodes (parents) into 8 contiguous ranges of N/8; each core owns
    the segment-sum + cell update for its parents.
  - Host does INDEX prep only: near-LPT assignment of parents to 512-parent
    blocks (bounds block edge count near the mean), within-block assignment of
    parents to four 128-slot windows (balances edges per window), per-core
    compacted child tables (for int16 dma_gather indices), one-hot P matrices
    for the segment-sum (index prep: P encodes edge->slot, values 0/1),
    pre-transposed fp16 x.
  - Device does all FLOP/memory work in fp16 (fp32 PSUM accumulation):
    batched dma_gather of child h||c rows, slot-windowed segment-sum on the
    PE (4 chunks of 128 edges -> 128-slot windows plus one 512-wide catch-all
    chunk for window overflow; ONE PSUM accumulation group per bank), dense
    LSTM matmuls, sigmoid/tanh on ACT, elementwise on DVE, fp16 output store.

Everything on device is computed in TRANSPOSED orientation [feature, node]:
  V[e, 0:128]=h_child, V[e,128:256]=c_child    (5*128 gathered edges/block)
  h_sumT[h, s] += V_h^T P_k                    (PE, chunk k; P from host)
  fT/iT/oT/uT[h, n] = W^T xT + U^T h_sumT      (PE; W/U natural layout as lhsT)
  biases ride the ACT `bias` operand (per-partition = per-feature).
Output is written transposed fp16 [256, npad] and de-transposed on host.

Engine placement notes (trn2):
  - No DVE tensor_scalar/cast ops in the loop: 2-port DVE perf-mode ops lock
    the SBUF port pair shared with GPSIMD and starve SWDGE descriptor
    generation for the gather (measured 7.3ms -> this design avoids it).
  - Plain DMAs ride HWDGE (nc.sync, SP queue), gather rides SWDGE (Pool).
  - DVE keeps only 1-port ops: tensor_tensor and PSUM-source tensor_copy.
"""

import os
import sys
import time

for _p in ("/opt/trn_rl_repo", "/root/.axon_site/_ro/trn_rl_repo"):
    if os.path.isdir(_p) and _p not in sys.path:
        sys.path.insert(0, _p)

import numpy as np

import concourse.bass as bass
import concourse.tile as tile
from concourse import mybir
from concourse.bass_utils import run_bass_kernel_spmd
from concourse.library_config import mlp as _mlp_library
from concourse.library_overlay import lower_extended_insts
from concourse.vector_clock import ScopedClock

CORES = 8
S = 512          # parents per block (= PSUM bank free dim in fp32)
P128 = 128
NW = 4           # slot windows per block (128 slots each)
CH = NW + 1      # gather chunks per block: 4 windowed + 1 catch-all
EPB = CH * P128  # padded edges per block (640)
SPB = EPB // 16  # int16 idx columns per block (40)
PCOLS = NW * P128 + S   # P matrix cols per block: 4x128 windows + 512 catch-all
MAX_IDX16 = 32000  # max rows per gather table (int16 indices)

F32 = mybir.dt.float32
F16 = mybir.dt.float16
I16 = mybir.dt.int16
AF = mybir.ActivationFunctionType
ALU = mybir.AluOpType

# ---------------------------------------------------------------------------
# Workarounds: the walrus build in this container accepts at most ONE sync
# wait per instruction. (a) chunk the Tile tail-drain waits onto nops;
# (b) post-pass that hoists extra waits of any instruction onto preceding
# NoOps on the same engine.
# ---------------------------------------------------------------------------

def _drain_and_barrier_chunked(self, tick_clock, wait_clock):
    probe = self.nc.sync.nop()
    wait_clock.add_sem_waits(probe.ins, ScopedClock({None: tick_clock.global_clock}))
    si = probe.ins.sync_info
    waits = list(si.on_wait) if si is not None else []
    if si is not None:
        probe.ins.sync_info = mybir.SyncInfo(on_wait=waits[:1], on_update=list(si.on_update))
    for i in range(1, len(waits)):
        nop = self.nc.sync.nop()
        nop.ins.sync_info = mybir.SyncInfo(on_wait=waits[i:i + 1], on_update=[])
    self.nc.sync.drain()
    self.nc.all_engine_barrier()
    popped = self.nc._tile_sem_poison_stack.pop()
    assert popped is self._sem_poison
    self.nc.clear_and_free_semaphores(list(self.sems.allocated().values()))
    self.nc.all_engine_barrier()


tile.TileContext._drain_and_barrier = _drain_and_barrier_chunked

_WSPLIT_CTR = [0]


def _split_multi_waits(nc):
    n_split = 0
    for f in nc.m.functions:
        for bb in f.blocks:
            insts = list(bb.instructions)
            if not any(
                i.sync_info is not None and i.sync_info.on_wait and len(i.sync_info.on_wait) > 1
                for i in insts
            ):
                continue
            new = []
            for inst in insts:
                si = inst.sync_info
                if si is not None and si.on_wait and len(si.on_wait) > 1:
                    waits = list(si.on_wait)
                    n_split += 1
                    for w in waits[:-1]:
                        _WSPLIT_CTR[0] += 1
                        new.append(
                            mybir.InstNoOp(
                                name=f"I-wsplit-{_WSPLIT_CTR[0]}",
                                engine=inst.engine,
                                debug=inst.debug,
                                ins=[],
                                outs=[],
                                sync_info=mybir.SyncInfo(on_wait=[w], on_update=[]),
                            )
                        )
                    inst.sync_info = mybir.SyncInfo(
                        on_wait=[waits[-1]], on_update=list(si.on_update)
                    )
                new.append(inst)
            bb.instructions = new
    return n_split


# ---------------------------------------------------------------------------
# Host-side index prep
# ---------------------------------------------------------------------------

def _prep(x, h, c, child_idx, parent_idx):
    N = x.shape[0]
    npc = (N + CORES - 1) // CORES            # parents per core
    nb = (npc + S - 1) // S                   # blocks per core
    npad = nb * S
    nbt = CORES * nb                          # total blocks

    parent = np.asarray(parent_idx).astype(np.int64)
    child = np.asarray(child_idx).astype(np.int64)

    # ---- near-LPT parent -> block assignment (bounds every block's edge
    # count near the mean; the relabeling is free: xT columns and output rows
    # are permuted on the host anyway).
    deg = np.bincount(parent, minlength=N)
    loads = np.zeros(nbt, np.int64)
    pcount = np.zeros(nbt, np.int64)
    gblock = np.empty(N, np.int64)
    for d in range(int(deg.max()), 0, -1):
        members = np.nonzero(deg == d)[0]
        if len(members) == 0:
            continue
        border = np.argsort(loads, kind="stable")
        k = len(members)
        slots_assign = np.tile(border, -(-k // nbt))[:k]
        gblock[members] = slots_assign
        loads += np.bincount(slots_assign, minlength=nbt) * d
        pcount += np.bincount(slots_assign, minlength=nbt)
    # zero-degree parents fill remaining slot capacity exactly
    d0 = np.nonzero(deg == 0)[0]
    cap = S - pcount
    fill = np.repeat(np.arange(nbt), cap)[: len(d0)]
    gblock[d0] = fill
    pcount += np.bincount(fill, minlength=nbt)
    assert pcount.max() <= S, pcount.max()
    assert loads.max() <= NW * P128 + P128, loads.max()

    # ---- within-block slot assignment: snake-deal parents (sorted by degree
    # desc) across the NW windows so every window's edge count is near the
    # mean; window overflow beyond 128 edges goes to the catch-all chunk.
    order_p = np.lexsort((np.arange(N), -deg, gblock))  # block asc, deg desc
    counts = np.bincount(gblock, minlength=nbt)
    starts = np.zeros(nbt + 1, np.int64)
    starts[1:] = np.cumsum(counts)
    rank_in_block = np.arange(N) - starts[gblock[order_p]]
    # snake pattern 0,1,..,NW-1,NW-1,..,1,0 repeating
    snake = np.concatenate([np.arange(NW), np.arange(NW)[::-1]])
    win_of_rank = snake[rank_in_block % (2 * NW)]
    # position within window = count of earlier ranks in same window
    pos_in_win = rank_in_block // (2 * NW) * 2
    first_pass = rank_in_block % (2 * NW) < NW
    pos_in_win = pos_in_win + (~first_pass).astype(np.int64)
    slot_of = np.empty(N, np.int64)
    slot_of[order_p] = win_of_rank * P128 + pos_in_win
    win_of = np.empty(N, np.int64)
    win_of[order_p] = win_of_rank

    # ---- edges sorted by (block, window, child): locality + chunking
    eblock = gblock[parent]
    ewin = win_of[parent]
    eorder = np.lexsort((child, ewin, eblock))
    se_block = eblock[eorder]
    se_win = ewin[eorder]
    se_child = child[eorder]
    se_slot = slot_of[parent][eorder]
    ebw_key = se_block * NW + se_win
    ebw_counts = np.bincount(ebw_key, minlength=nbt * NW)
    ebw_starts = np.zeros(nbt * NW + 1, np.int64)
    ebw_starts[1:] = np.cumsum(ebw_counts)

    hc = np.concatenate([np.asarray(h), np.asarray(c)], axis=1).astype(np.float16)

    # per-core arrays
    core_child = []     # [nb*EPB] gather child (global id), padding=0
    core_Ps = []        # [128, nb*PCOLS] fp16 one-hot
    max_ca = 0
    for i in range(CORES):
        gidx = np.zeros(nb * EPB, np.int64)
        P = np.zeros((P128, nb * PCOLS), np.float16)
        for b in range(nb):
            gb = i * nb + b
            ca_ch = []
            ca_sl = []
            for w in range(NW):
                e0, e1 = ebw_starts[gb * NW + w], ebw_starts[gb * NW + w + 1]
                mm = e1 - e0
                take = min(mm, P128)
                ch = se_child[e0:e0 + take]
                sl = se_slot[e0:e0 + take] - w * P128
                off = b * EPB + w * P128
                gidx[off:off + take] = ch
                # edge linear index off+j -> partition (off+j) % 128 , chunk w
                P[np.arange(take), b * PCOLS + w * P128 + sl] = 1.0
                if mm > take:
                    ca_ch.extend(se_child[e0 + take:e1])
                    ca_sl.extend(se_slot[e0 + take:e1])
            nca = len(ca_ch)
            assert nca <= P128, f"catch-all overflow {nca} in core {i} block {b}"
            max_ca = max(max_ca, nca)
            if nca:
                off = b * EPB + NW * P128
                gidx[off:off + nca] = ca_ch
                P[np.arange(nca), b * PCOLS + NW * P128 + np.asarray(ca_sl)] = 1.0
        core_child.append(gidx)
        core_Ps.append(P)

    # ---- split blocks into G groups so every per-core group table fits int16
    G = 1
    while True:
        nbg = (nb + G - 1) // G
        ok = True
        groups = []  # per core: list of (b0, b1, uniq, remapped idx)
        u_max = [0] * G
        for i in range(CORES):
            gi = []
            for g in range(G):
                b0, b1 = g * nbg, min((g + 1) * nbg, nb)
                seg = slice(b0 * EPB, b1 * EPB)
                ch = core_child[i][seg]
                uniq = np.unique(ch)
                if len(uniq) > MAX_IDX16:
                    ok = False
                    break
                rem = np.searchsorted(uniq, ch)
                u_max[g] = max(u_max[g], len(uniq))
                gi.append((b0, b1, uniq, rem))
            if not ok:
                break
            groups.append(gi)
        if ok:
            break
        G += 1

    core_of_p = gblock // nb
    col_of = (gblock % nb) * S + slot_of
    in_maps = []
    assembly = []
    xx = np.asarray(x)
    for i in range(CORES):
        im = {}
        idx16 = np.zeros((P128, nb * SPB), np.int16)
        for g in range(G):
            b0, b1, uniq, rem = groups[i][g]
            tbl = np.zeros((u_max[g], 256), np.float16)
            tbl[: len(uniq)] = hc[uniq]
            im[f"hc{g}"] = tbl
            for b in range(b0, b1):
                e = rem[(b - b0) * EPB:(b - b0 + 1) * EPB].astype(np.int16)
                # dma_gather order: edge i = col*16 + partition(p<16); replicate x8
                blk = e.reshape(SPB, 16).T          # [16, SPB]
                idx16[:, b * SPB:(b + 1) * SPB] = np.tile(blk, (8, 1))
        im["idx16"] = idx16
        pi = np.nonzero(core_of_p == i)[0]
        cols = col_of[pi]
        xT = np.zeros((P128, npad), np.float16)
        xT[:, cols] = xx[pi].T.astype(np.float16)
        # merge xT + P into one per-block stream: [512 xT | 1024 P] per block
        # (one HWDGE DMA per block instead of two — SP sequencer time is a
        # serial ~565ns per dma_start).
        xp = np.empty((P128, nb * (S + PCOLS)), np.float16)
        xpv = xp.reshape(P128, nb, S + PCOLS)
        xpv[:, :, :S] = xT.reshape(P128, nb, S)
        xpv[:, :, S:] = core_Ps[i].reshape(P128, nb, PCOLS)
        im["xp"] = xp
        in_maps.append(im)
        assembly.append((pi, cols))

    return in_maps, assembly, nb, npad, [int(u) for u in u_max], G


# ---------------------------------------------------------------------------
# Device program
# ---------------------------------------------------------------------------

def _build_nc(nb, npad, u_max, G, repeat=1):
    """repeat>1 wraps the whole block loop in a HW For_i — used only by the
    benchmark to amortize the ~78ms axon dispatch RTT over R executions."""
    nbg = (nb + G - 1) // G

    nc = bass.Bass("TRN2", target_bir_lowering=False, debug=False)
    xp_t = nc.dram_tensor("xp", [P128, nb * (S + PCOLS)], F16, kind="ExternalInput")
    hc_ts = [
        nc.dram_tensor(f"hc{g}", [u_max[g], 256], F16, kind="ExternalInput")
        for g in range(G)
    ]
    idx_t = nc.dram_tensor("idx16", [P128, nb * SPB], I16, kind="ExternalInput")
    wf_t = nc.dram_tensor("W_f", [128, 128], F16, kind="ExternalInput")
    uf_t = nc.dram_tensor("U_f", [128, 128], F16, kind="ExternalInput")
    wio_t = nc.dram_tensor("W_iou", [128, 384], F16, kind="ExternalInput")
    uio_t = nc.dram_tensor("U_iou", [128, 384], F16, kind="ExternalInput")
    bf_t = nc.dram_tensor("b_f", [128, 1], F32, kind="ExternalInput")
    bio_t = nc.dram_tensor("b_iou", [384, 1], F32, kind="ExternalInput")
    out_t = nc.dram_tensor("outT", [256, npad], F16, kind="ExternalOutput")

    # Emit the library load BEFORE the TileContext so it sits at the head of
    # the basic block, ahead of every Tile-scheduled Pool instruction — the
    # scheduler has no data dep to order it before the dma_gathers otherwise.
    nc.gpsimd.load_library(_mlp_library)  # dma_gather Q7 ucode

    with tile.TileContext(nc) as tc:
        with (
            tc.tile_pool(name="const", bufs=1) as cpool,
            tc.tile_pool(name="vpool", bufs=3) as vpool,
            tc.tile_pool(name="xpool", bufs=3) as xpool,
            tc.tile_pool(name="hpool", bufs=2) as hpool,
            tc.tile_pool(name="fpool", bufs=2) as fpool,
            tc.tile_pool(name="gpool", bufs=2) as gpool,
            tc.tile_pool(name="spool", bufs=2) as spool,   # i sig
            tc.tile_pool(name="opool", bufs=2) as opool,   # o sig
            tc.tile_pool(name="upool", bufs=2) as upool,   # u tanh
            tc.tile_pool(name="npool", bufs=3) as npool,   # c_new
            tc.tile_pool(name="tpool", bufs=2) as tpool,   # tanh(c_new)
            tc.tile_pool(name="ypool", bufs=3) as ypool,   # h_new
            tc.tile_pool(name="psA", bufs=2, space="PSUM") as psA,
            tc.tile_pool(name="psB", bufs=2, space="PSUM") as psB,
            tc.tile_pool(name="psF", bufs=1, space="PSUM") as psF,
            tc.tile_pool(name="psIO", bufs=3, space="PSUM") as psIO,
        ):
            wf_sb = cpool.tile([128, 128], F16)
            nc.sync.dma_start(out=wf_sb[:], in_=wf_t[:, :])
            uf_sb = cpool.tile([128, 128], F16)
            nc.sync.dma_start(out=uf_sb[:], in_=uf_t[:, :])
            wio_sb = cpool.tile([128, 384], F16)
            nc.sync.dma_start(out=wio_sb[:], in_=wio_t[:, :])
            uio_sb = cpool.tile([128, 384], F16)
            nc.sync.dma_start(out=uio_sb[:], in_=uio_t[:, :])
            idx_sb = cpool.tile([P128, nb * SPB], I16)
            nc.sync.dma_start(out=idx_sb[:], in_=idx_t[:, :])
            bf_col = cpool.tile([128, 1], F32)
            nc.sync.dma_start(out=bf_col[:], in_=bf_t[:, :])
            bio_cols = []
            for t in range(3):
                col = cpool.tile([128, 1], F32, tag=f"bio_col{t}")
                nc.sync.dma_start(out=col[:], in_=bio_t[t * 128:(t + 1) * 128, :])
                bio_cols.append(col)
            epb_reg = nc.gpsimd.to_reg(EPB)  # shared num_idxs register

            from contextlib import ExitStack as _ES
            _loop_ctx = _ES()
            if repeat > 1:
                _loop_ctx.enter_context(tc.For_i(0, repeat, 1))
            BPC = S + PCOLS
            for b in range(nb):
                g = b // nbg
                xp_sb = xpool.tile([128, BPC], F16)
                nc.sync.dma_start(out=xp_sb[:], in_=xp_t[:, b * BPC:(b + 1) * BPC])
                xT_sb = xp_sb[:, 0:S]
                P_sb = xp_sb[:, S:BPC]

                # one batched gather for the whole block's padded edges
                Vall = vpool.tile([128, CH * 256], F16)
                nc.gpsimd.dma_gather(
                    out_ap=Vall[:].rearrange("p (c e) -> p c e", e=256),
                    in_ap=hc_ts[g][:, :],
                    idxs_ap=idx_sb[:, b * SPB:(b + 1) * SPB],
                    num_idxs=EPB,
                    num_idxs_reg=epb_reg,
                    elem_size=256,
                )

                # slot-windowed segment sum. Order-independent formulation:
                # the catch-all chunk goes FIRST with start=True — it clears
                # the whole 2KB zero region AND writes all 512 columns (sets
                # every has_written bit), so each window matmul is a pure
                # accumulate into its 128-col region. The windows' mutual
                # order doesn't matter, and each has a WAW dep on the
                # catch-all (overlapping region), so Tile can't hoist them
                # above it. stop on the last-emitted window is sim-only
                # bookkeeping (no HW effect); skip_group_check because the
                # scheduler may run windows in any order.
                ps_hT = psA.tile([128, S], F32, space="PSUM")
                ps_cT = psB.tile([128, S], F32, space="PSUM")
                nc.tensor.matmul(
                    out=ps_hT[:], lhsT=Vall[:, NW * 256:NW * 256 + 128],
                    rhs=P_sb[:, NW * P128:NW * P128 + S],
                    start=True, stop=False,
                )
                for k in range(NW):
                    nc.tensor.matmul(
                        out=ps_hT[:, k * P128:(k + 1) * P128],
                        lhsT=Vall[:, k * 256:k * 256 + 128],
                        rhs=P_sb[:, k * P128:(k + 1) * P128],
                        start=False, stop=(k == NW - 1),
                        skip_group_check=(k != NW - 1),
                    )
                nc.tensor.matmul(
                    out=ps_cT[:], lhsT=Vall[:, NW * 256 + 128:(NW + 1) * 256],
                    rhs=P_sb[:, NW * P128:NW * P128 + S],
                    start=True, stop=False,
                )
                for k in range(NW):
                    nc.tensor.matmul(
                        out=ps_cT[:, k * P128:(k + 1) * P128],
                        lhsT=Vall[:, k * 256 + 128:(k + 1) * 256],
                        rhs=P_sb[:, k * P128:(k + 1) * P128],
                        start=False, stop=(k == NW - 1),
                        skip_group_check=(k != NW - 1),
                    )

                hsumT_sb = hpool.tile([128, S], F16)
                nc.vector.tensor_copy(out=hsumT_sb[:], in_=ps_hT[:])

                ps_fT = psF.tile([128, S], F32, space="PSUM")
                nc.tensor.matmul(out=ps_fT[:], lhsT=wf_sb[:], rhs=xT_sb,
                                 start=True, stop=False)
                nc.tensor.matmul(out=ps_fT[:], lhsT=uf_sb[:], rhs=hsumT_sb[:],
                                 start=False, stop=True)
                fsig = fpool.tile([128, S], F16)
                nc.scalar.activation(out=fsig[:], in_=ps_fT[:], func=AF.Sigmoid,
                                     bias=bf_col[:])
                caggT = gpool.tile([128, S], F16)
                nc.vector.tensor_tensor(out=caggT[:], in0=fsig[:], in1=ps_cT[:],
                                        op=ALU.mult)

                iou_sb = []
                for t, func in ((0, AF.Sigmoid), (1, AF.Sigmoid), (2, AF.Tanh)):
                    ps = psIO.tile([128, S], F32, space="PSUM")
                    nc.tensor.matmul(out=ps[:], lhsT=wio_sb[:, t * 128:(t + 1) * 128],
                                     rhs=xT_sb, start=True, stop=False)
                    nc.tensor.matmul(out=ps[:], lhsT=uio_sb[:, t * 128:(t + 1) * 128],
                                     rhs=hsumT_sb[:], start=False, stop=True)
                    dest = (spool, opool, upool)[t].tile([128, S], F16)
                    nc.scalar.activation(out=dest[:], in_=ps[:], func=func,
                                         bias=bio_cols[t][:])
                    iou_sb.append(dest)
                isig, osig, utanh = iou_sb

                cnew = npool.tile([128, S], F16)
                nc.vector.tensor_tensor(out=cnew[:], in0=isig[:], in1=utanh[:],
                                        op=ALU.mult)
                nc.vector.tensor_tensor(out=cnew[:], in0=cnew[:], in1=caggT[:],
                                        op=ALU.add)
                tanhc = tpool.tile([128, S], F16)
                nc.scalar.activation(out=tanhc[:], in_=cnew[:], func=AF.Tanh)
                hnew = ypool.tile([128, S], F16)
                nc.vector.tensor_tensor(out=hnew[:], in0=osig[:], in1=tanhc[:],
                                        op=ALU.mult)
                nc.sync.dma_start(out=out_t[0:128, b * S:(b + 1) * S], in_=hnew[:])
                nc.sync.dma_start(out=out_t[128:256, b * S:(b + 1) * S], in_=cnew[:])
            _loop_ctx.close()

    _split_multi_waits(nc)
    lower_extended_insts(nc)  # populate .instr for InstPseudoReloadLibraryIndex
    return nc


LAST_EXEC_TIME_NS = None
LAST_RESULTS = None
_LAST_RUN = None  # (nc, in_maps) for benchmarking


def _make_runner(nc, in_maps, warmup=2):
    """Compile + stage a NEFF executor with device-resident inputs; returns a
    zero-arg callable measuring one blocking execute (wall seconds)."""
    import jax
    from jax.sharding import Mesh, PartitionSpec, NamedSharding
    try:
        from jax.experimental.shard_map import shard_map
    except ImportError:
        from jax.shard_map import shard_map
    from concourse import bass2jax

    bass2jax.install_neuronx_cc_hook()
    n_cores = len(in_maps)

    partition_name = nc.partition_id_tensor.name if nc.partition_id_tensor else None
    in_names, out_names, out_avals, zero_outs = [], [], [], []
    for alloc in nc.m.functions[0].allocations:
        if not isinstance(alloc, mybir.MemoryLocationSet):
            continue
        name = alloc.memorylocations[0].name
        if alloc.kind == "ExternalInput":
            if name != partition_name:
                in_names.append(name)
        elif alloc.kind == "ExternalOutput":
            shape = tuple(alloc.tensor_shape)
            dtype = mybir.dt.np(alloc.dtype)
            out_names.append(name)
            out_avals.append(jax.core.ShapedArray(shape, dtype))
            zero_outs.append(np.zeros(shape, dtype))
    n_params = len(in_names)
    all_names = in_names + out_names
    if partition_name is not None:
        all_names = all_names + [partition_name]

    def _body(*args):
        operands = list(args)
        if partition_name is not None:
            operands.append(bass2jax.partition_id_tensor())
        outs = bass2jax._bass_exec_p.bind(
            *operands,
            out_avals=tuple(out_avals),
            in_names=tuple(all_names),
            out_names=tuple(out_names),
            lowering_input_output_aliases=(),
            sim_require_finite=True,
            sim_require_nnan=True,
            nc=nc,
        )
        return tuple(outs)

    devices = jax.devices()[:n_cores]
    mesh = Mesh(np.asarray(devices), ("core",))
    spec = PartitionSpec("core")
    fn = jax.jit(
        shard_map(
            _body, mesh=mesh,
            in_specs=(spec,) * (n_params + len(out_names)),
            out_specs=(spec,) * len(out_names),
            check_rep=False,
        ),
        keep_unused=True,
    )
    sh = NamedSharding(mesh, spec)
    args = [
        jax.device_put(
            np.concatenate([np.asarray(in_maps[c][nm]) for c in range(n_cores)], axis=0), sh
        )
        for nm in in_names
    ] + [
        jax.device_put(np.concatenate([z] * n_cores, axis=0), sh) for z in zero_outs
    ]

    for _ in range(warmup):
        out = fn(*args)
    jax.block_until_ready(out)

    def call():
        t0 = time.perf_counter()
        out = fn(*args)
        jax.block_until_ready(out)
        return time.perf_counter() - t0

    return call


_LAST_BUILD_ARGS = None


def benchmark_last(iters=24, reps=8):
    """Device-time estimate that defeats the ~60-80ms axon dispatch RTT (and
    its drift): build a variant of the same kernel whose block loop runs
    `reps` times inside a HW For_i (the kernel is idempotent), INTERLEAVE
    R=1 / R=reps calls so network drift cancels, then
      device_ns = (min_call(R=reps) - min_call(R=1)) / (reps - 1)."""
    global LAST_EXEC_TIME_NS
    assert _LAST_RUN is not None, "call kernel() first"
    nc1, in_maps = _LAST_RUN
    nb, npad, u_max, G = _LAST_BUILD_ARGS
    nc_r = _build_nc(nb, npad, u_max, G, repeat=reps)
    call1 = _make_runner(nc1, in_maps)
    callr = _make_runner(nc_r, in_maps)
    t1 = tr = float("inf")
    for _ in range(iters):
        t1 = min(t1, call1())
        tr = min(tr, callr())
    t1, tr = int(t1 * 1e9), int(tr * 1e9)
    dev = int((tr - t1) / (reps - 1))
    print(f"  [bench] min per-call: R=1: {t1} ns, R={reps}: {tr} ns"
          f" -> device ~{dev} ns/exec")
    LAST_EXEC_TIME_NS = dev if dev > 0 else t1
    return LAST_EXEC_TIME_NS


def kernel(x, h, c, child_idx, parent_idx, W_f, U_f, b_f, W_iou, U_iou, b_iou,
           trace=False, trace_cores=None):
    global LAST_EXEC_TIME_NS, LAST_RESULTS, _LAST_RUN, _LAST_BUILD_ARGS
    x = np.asarray(x, np.float32)
    N = x.shape[0]
    in_maps, assembly, nb, npad, u_max, G = _prep(
        x, h, c, child_idx, parent_idx)

    _LAST_BUILD_ARGS = (nb, npad, u_max, G)
    nc = _build_nc(nb, npad, u_max, G)
    for im in in_maps:
        im["W_f"] = np.asarray(W_f, np.float16)
        im["U_f"] = np.asarray(U_f, np.float16)
        im["W_iou"] = np.asarray(W_iou, np.float16)
        im["U_iou"] = np.asarray(U_iou, np.float16)
        im["b_f"] = np.asarray(b_f, np.float32).reshape(128, 1)
        im["b_iou"] = np.asarray(b_iou, np.float32).reshape(384, 1)

    kwargs = {}
    if trace:
        kwargs["trace"] = True
        if trace_cores is not None:
            kwargs["trace_cores"] = trace_cores

    for attempt in range(3):
        res = run_bass_kernel_spmd(nc, in_maps, core_ids=list(range(CORES)), **kwargs)
        LAST_EXEC_TIME_NS = res.exec_time_ns
        LAST_RESULTS = res
        _LAST_RUN = (nc, in_maps)
        out = np.empty((N, 256), np.float32)
        for i, (pi, cols) in enumerate(assembly):
            out[pi] = res.results[i]["outT"].T[cols].astype(np.float32)
        err = _sample_check(out, x, np.asarray(h), np.asarray(c),
                            np.asarray(child_idx), np.asarray(parent_idx),
                            W_f, U_f, b_f, W_iou, U_iou, b_iou)
        if err < 1e-2:
            break
        print(f"  [kernel] sample self-check failed (rel {err:.3e}); "
              f"retrying (device flake?)")
    return out


def _sample_check(out, x, h, c, child_idx, parent_idx,
                  W_f, U_f, b_f, W_iou, U_iou, b_iou, k=64):
    """Spot-check k random nodes against a numpy reference; catches silent
    device flakes (observed once: garbage output with no runtime error)."""
    rng = np.random.default_rng(0)
    nodes = rng.choice(x.shape[0], size=min(k, x.shape[0]), replace=False)
    sel = {int(n): i for i, n in enumerate(nodes)}
    hs = np.zeros((len(nodes), 128), np.float64)
    cs = np.zeros((len(nodes), 128), np.float64)
    m = np.isin(parent_idx, nodes)
    for p, ch in zip(parent_idx[m], child_idx[m]):
        i = sel[int(p)]
        hs[i] += h[ch]
        cs[i] += c[ch]
    xs = x[nodes].astype(np.float64)

    def sig(v):
        return 1.0 / (1.0 + np.exp(-v))

    f = sig(xs @ W_f + hs @ U_f + np.asarray(b_f))
    iou = xs @ W_iou + hs @ U_iou + np.asarray(b_iou)
    i_, o, u = np.split(iou, 3, axis=1)
    cn = sig(i_) * np.tanh(u) + f * cs
    hn = sig(o) * np.tanh(cn)
    exp = np.concatenate([hn, cn], axis=1)
    return float(np.abs(out[nodes] - exp).max() / max(1e-9, np.abs(exp).max()))
